# revision 1
# baseline (speedup 1.0000x reference)
"""Trainium2 Bass kernel for nn_MixedDecoder (moe_routing).

Math (matches the reference exactly): only the LAST expert layer matters —
the reference never feeds layer outputs back into `z`, so layers 0/1 are
dead code.  Computed per sample b:
    coef = softmax(gate_mlp(z))                        # [B, 8]
    out  = sum_e coef[b,e] * (z @ w2[e] + b2[e])       # [B, 256]

Sharding: data-parallel over batch B=2048 across 8 cores (256 rows/core),
weights replicated.  All matmul operands are bf16 (fp32 PSUM accumulation;
~5e-3 rel err, tolerance 2e-2) which halves HBM traffic vs fp32.  The
expert bias b2 is folded into the expert matmul via a constant-1 row
appended to z (K=289).  K is split 128+128+33; the 33-row tails (z and
w2) are packed into partitions 0:33 / 64:97 of otherwise-dead corners of
the two input tensors, so every DMA moves a dense [128, N] rectangle:
`zgp` carries zT passes + the whole gate MLP + the w2 K-tails; `w2p`
carries only the two full-height K-passes of the 4 expert pairs.

ELU is computed as relu(x)+min(exp(x),1) (monotonicity folds the min into
the exp) with the "+1" offset folded into adjusted next-layer biases.
Expert matmuls keep zT chunks stationary with expert pairs side-by-side as
a [K,512] moving operand.  The softmax-normalized coefficients (one cheap
[128,8] scale after reciprocal_approx_fast) scale the PSUM pair halves on
eviction: ACT evicts the even expert, DVE fuses the odd expert's scale and
the add in one scalar_tensor_tensor op; GPSIMD and DVE fold the 4 partial
sums (bf16 intermediates for 2x element rate).  Constants and PE warm-up
are hoisted out of the rep body; tile pools are double-buffered so
consecutive executions overlap.
"""

import numpy as np
import ml_dtypes

N_CORES = 8
B = 2048
IN_SIZE = 288
E = 8
GATE_H = 64
OUT_SIZE = 256
BL = B // N_CORES          # 256 rows per core
NCH = BL // 128            # 2 batch chunks of 128
NP = 4                     # expert pairs

# zg column layout
G0 = 3 * BL                # 768: g0_w passes
G1 = G0 + 3 * GATE_H       # 960: g1_w
G2 = G1 + GATE_H           # 1024: g2_w
ADJ = G2 + E               # 1032: adj2 row (partition 0)
B0 = ADJ + E               # 1040: g0_b column (rows 0:64)
B1 = B0 + 1                # 1041: b1_adj column
WT = B1 + 1                # 1042: w2 K-tail blocks (2 x 512 cols)
ZGC = WT + 2 * 512         # 2066 total zg cols

_CACHE = {}


def _build_nc(reps=1):
    from concourse import bacc
    import concourse.mybir as mybir
    from concourse.tile import TileContext

    dt = mybir.dt
    F32 = dt.float32
    BF16 = dt.bfloat16
    AF = mybir.ActivationFunctionType
    OP = mybir.AluOpType

    nc = bacc.Bacc("TRN2", target_bir_lowering=False, debug=False)

    zg_d = nc.declare_dram_parameter("zgp", [128, ZGC], BF16, isOutput=False)
    w2_d = nc.declare_dram_parameter("w2p", [128, NP * 1024], BF16,
                                     isOutput=False)
    out_d = nc.declare_dram_parameter("outp", [128, NCH * OUT_SIZE], BF16,
                                      isOutput=True)

    with TileContext(nc) as tc:
        with (
            tc.tile_pool(name="const", bufs=1) as cp,
            tc.tile_pool(name="zg", bufs=3) as zp,
            tc.tile_pool(name="w2", bufs=2) as wp,
            tc.tile_pool(name="wk", bufs=2) as wk,
            tc.tile_pool(name="py", bufs=5, space="PSUM") as py,
            tc.tile_pool(name="pg", bufs=3, space="PSUM") as pg,
        ):
            # ---- once-only prologue: constants, ACT table warm, PE warm ----
            wz = cp.tile([128, 128], BF16, name="wz")
            nc.vector.memset(wz[:], 0.0)
            warm = cp.tile([1, 1], F32, name="warm")
            nc.vector.memset(warm[:], 0.0)
            warm2 = cp.tile([1, 1], F32, name="warm2")
            nc.scalar.activation(warm2[:], warm[:], AF.Exp)
            wu_ps = py.tile([128, 512], F32, name="wups", tag="py")
            for _ in range(8):
                nc.tensor.matmul(wu_ps[:, 0:128], wz[:], wz[:],
                                 start=True, stop=True)

            for _rep in range(reps):
                zg = zp.tile([128, ZGC], BF16, name="zg")
                # split at the w2-tail boundary: the gate only needs part 1,
                # so it can start while the tails stream in
                nc.sync.dma_start(out=zg[:, 0:WT], in_=zg_d.ap()[:, 0:WT])
                nc.sync.dma_start(out=zg[:, WT:ZGC], in_=zg_d.ap()[:, WT:ZGC])
                w2r = wp.tile([128, NP * 1024], BF16, name="w2r")
                for p in range(NP):
                    nc.sync.dma_start(out=w2r[:, p * 1024:(p + 1) * 1024],
                                      in_=w2_d.ap()[:, p * 1024:(p + 1) * 1024])

                # engine scalar operands must be f32: upconvert the two
                # bf16 bias columns once (cheaper than a separate DMA)
                bias32 = wk.tile([GATE_H, 2], F32, name="bias32")
                nc.vector.tensor_copy(bias32[:], zg[0:GATE_H, B0:B0 + 2])
                g0b = bias32[:, 0:1]
                b1_adj = bias32[:, 1:2]

                def elu_pieces(ps_in, bias, pref, ones_row=False):
                    # elu(x)+1 as two summable pieces: relu(x), min(exp(x),1).
                    # ones_row appends a constant-1 row 64 to the relu piece so
                    # a consumer matmul at K=65 picks up a folded bias row.
                    t_exp = wk.tile([GATE_H, BL], F32, name=f"{pref}_exp")
                    nc.scalar.activation(t_exp[:], ps_in, AF.Exp, bias=bias)
                    t_min = wk.tile([GATE_H, BL], BF16, name=f"{pref}_min")
                    nc.vector.tensor_scalar(t_min[:], t_exp[:], 1.0, None, OP.min)
                    rows = GATE_H + 1 if ones_row else GATE_H
                    t_relu = wk.tile([rows, BL], BF16, name=f"{pref}_relu")
                    nc.vector.tensor_scalar(t_relu[0:GATE_H, :], ps_in, bias, 0.0,
                                            OP.add, OP.max)
                    if ones_row:
                        nc.vector.memset(t_relu[GATE_H:GATE_H + 1, :], 1.0)
                    return t_relu, t_min

                with tc.high_priority():
                    h0_ps = pg.tile([GATE_H, BL], F32, name="h0ps", tag="pg")
                    nc.tensor.matmul(h0_ps[:], zg[0:128, G0:G0 + GATE_H],
                                     zg[0:128, 0:BL], start=True, stop=False)
                    nc.tensor.matmul(h0_ps[:], zg[0:128, G0 + GATE_H:G0 + 2 * GATE_H],
                                     zg[0:128, BL:2 * BL], start=False, stop=False)
                    nc.tensor.matmul(h0_ps[:], zg[64:97, G0 + 2 * GATE_H:G0 + 3 * GATE_H],
                                     zg[64:97, 2 * BL:3 * BL], start=False, stop=True)
                    h0_a, h0_b = elu_pieces(h0_ps[:], g0b, "e0")

                    h1_ps = pg.tile([GATE_H, BL], F32, name="h1ps", tag="pg")
                    g1w = zg[0:GATE_H, G1:G1 + GATE_H]
                    nc.tensor.matmul(h1_ps[:], g1w, h0_a[:], start=True, stop=False)
                    nc.tensor.matmul(h1_ps[:], g1w, h0_b[:], start=False, stop=True)
                    h1_a, h1_b = elu_pieces(h1_ps[:], b1_adj, "e1",
                                            ones_row=True)

                    # softmax-normalized coefficients per chunk: [128, 8]
                    expn = []
                    for c in range(NCH):
                        lg_ps = pg.tile([128, E], F32, name="lgps", tag="pg")
                        nc.tensor.matmul(lg_ps[:], h1_a[:, c * 128:(c + 1) * 128],
                                         zg[0:GATE_H + 1, G2:G2 + E],
                                         start=True, stop=False)
                        nc.tensor.matmul(lg_ps[:], h1_b[:, c * 128:(c + 1) * 128],
                                         zg[0:GATE_H, G2:G2 + E],
                                         start=False, stop=True)
                        expc = wk.tile([128, E], F32, name="expc")
                        sume = wk.tile([128, 1], F32, name="sume")
                        nc.scalar.activation(expc[:], lg_ps[:], AF.Exp,
                                             accum_out=sume[:])
                        rcp = wk.tile([128, 1], F32, name="rcp")
                        nc.vector.reciprocal_approx_fast(rcp[:], sume[:])
                        en = wk.tile([128, E], F32, name="expn")
                        nc.vector.tensor_scalar(en[:], expc[:], rcp[:], None,
                                                OP.mult)
                        expn.append(en)

                # ---------------- expert layer + combine ----------------
                out_sb = wk.tile([128, NCH * OUT_SIZE], BF16, name="outsb")
                for c in range(NCH):
                    accs = []
                    for p in range(NP):
                        yp = py.tile([128, 2 * OUT_SIZE], F32, name=f"yp{p}",
                                     tag="py")
                        nc.tensor.matmul(yp[:], zg[0:128, c * 128:c * 128 + 128],
                                         w2r[0:128, p * 1024:p * 1024 + 512],
                                         start=True, stop=False)
                        nc.tensor.matmul(yp[:], zg[0:128, BL + c * 128:BL + c * 128 + 128],
                                         w2r[0:128, p * 1024 + 512:p * 1024 + 1024],
                                         start=False, stop=False)
                        q = (p % 2) * 64      # K-tail parity row base
                        nc.tensor.matmul(yp[:], zg[q:q + 33, 2 * BL + c * 128:2 * BL + (c + 1) * 128],
                                         zg[q:q + 33, WT + (p // 2) * 512:WT + (p // 2) * 512 + 512],
                                         start=False, stop=True)
                        sa = wk.tile([128, OUT_SIZE], BF16, name=f"sa{p}")
                        nc.scalar.activation(sa[:], yp[:, 0:OUT_SIZE], AF.Copy,
                                             scale=expn[c][:, 2 * p:2 * p + 1])
                        acc = wk.tile([128, OUT_SIZE], BF16, name=f"acc{p}")
                        nc.vector.scalar_tensor_tensor(
                            acc[:], yp[:, OUT_SIZE:2 * OUT_SIZE],
                            expn[c][:, 2 * p + 1:2 * p + 2], sa[:],
                            OP.mult, OP.add)
                        accs.append(acc)
                    b01 = wk.tile([128, OUT_SIZE], BF16, name="b01")
                    nc.gpsimd.tensor_tensor(b01[:], accs[0][:], accs[1][:], OP.add)
                    b23 = wk.tile([128, OUT_SIZE], BF16, name="b23")
                    nc.gpsimd.tensor_tensor(b23[:], accs[2][:], accs[3][:], OP.add)
                    nc.gpsimd.tensor_tensor(
                        out_sb[:, c * OUT_SIZE:(c + 1) * OUT_SIZE],
                        b01[:], b23[:], OP.add)
                # SWDGE via gpsimd: the out DMA issues right after the final
                # adds on the same engine, so it never head-of-line-blocks the
                # SP HWDGE ring that streams the next rep's inputs
                nc.gpsimd.dma_start(out=out_d.ap(), in_=out_sb[:])

    nc.finalize()
    return nc


def _get_nc(reps=1):
    key = ("nc", reps)
    if key not in _CACHE:
        _CACHE[key] = _build_nc(reps)
    return _CACHE[key]


def _bf(x):
    return np.ascontiguousarray(np.asarray(x, np.float32)).astype(
        ml_dtypes.bfloat16)


def make_in_maps(z, g0_w, g0_b, g1_w, g1_b, g2_w, g2_b, w2, b2, **_unused):
    z = np.asarray(z, dtype=np.float32)
    g0_w = np.asarray(g0_w, dtype=np.float32)
    g1_w = np.asarray(g1_w, dtype=np.float32)
    g2_w = np.asarray(g2_w, dtype=np.float32)
    g0_b = np.asarray(g0_b, dtype=np.float32)
    g1_b = np.asarray(g1_b, dtype=np.float32)
    g2_b = np.asarray(g2_b, dtype=np.float32)
    w2 = np.asarray(w2, dtype=np.float32)
    b2 = np.asarray(b2, dtype=np.float32)

    # shared block of zg (cols G0:ZGC): gate MLP weights + w2 K-tails
    gshared = np.zeros((128, ZGC - G0), dtype=np.float32)
    gshared[0:128, 0:GATE_H] = g0_w[0:128]
    gshared[0:128, GATE_H:2 * GATE_H] = g0_w[128:256]
    gshared[64:96, 2 * GATE_H:3 * GATE_H] = g0_w[256:288]
    gshared[0:GATE_H, G1 - G0:G1 - G0 + GATE_H] = g1_w
    gshared[0:GATE_H, G2 - G0:G2 - G0 + E] = g2_w
    gshared[GATE_H, G2 - G0:G2 - G0 + E] = g2_b - g2_w.sum(axis=0)
    gshared[0:GATE_H, B0 - G0] = g0_b
    # adjusted bias absorbs the ELU "+1" offset of the previous layer
    gshared[0:GATE_H, B1 - G0] = g1_b - g1_w.sum(axis=0)
    for p in range(NP):
        pair_w = np.concatenate([w2[2 * p], w2[2 * p + 1]], axis=1)  # [288,512]
        pair_b = np.concatenate([b2[2 * p], b2[2 * p + 1]])          # [512]
        q = (p % 2) * 64
        c0 = WT - G0 + (p // 2) * 512
        gshared[q:q + 32, c0:c0 + 512] = pair_w[256:288]
        gshared[q + 32, c0:c0 + 512] = pair_b

    # w2 full-height K-passes: [128, 4*1024]
    w2p = np.zeros((128, NP * 1024), dtype=np.float32)
    for p in range(NP):
        pair_w = np.concatenate([w2[2 * p], w2[2 * p + 1]], axis=1)
        w2p[:, p * 1024:p * 1024 + 512] = pair_w[0:128]
        w2p[:, p * 1024 + 512:(p + 1) * 1024] = pair_w[128:256]

    shared = {"w2p": _bf(w2p)}
    gsh_bf = gshared
    maps = []
    for c in range(N_CORES):
        zT = z[c * BL:(c + 1) * BL].T                      # [288, 256]
        zgp = np.zeros((128, ZGC), dtype=np.float32)
        zgp[0:128, 0:BL] = zT[0:128]
        zgp[0:128, BL:2 * BL] = zT[128:256]
        # K-tail of z (+ the constant-1 bias row) duplicated at both parity
        # bases so either w2-tail parity block has a matching stationary
        zgp[0:32, 2 * BL:3 * BL] = zT[256:288]
        zgp[32, 2 * BL:3 * BL] = 1.0
        zgp[64:96, 2 * BL:3 * BL] = zT[256:288]
        zgp[96, 2 * BL:3 * BL] = 1.0
        zgp[:, G0:] = gsh_bf
        maps.append(dict(shared, zgp=_bf(zgp)))
    return maps


def unpack_out(res_list):
    full = np.empty((B, OUT_SIZE), dtype=np.float32)
    for c in range(N_CORES):
        packed = np.asarray(res_list[c]["outp"], dtype=np.float32)
        for ch in range(NCH):
            full[c * BL + ch * 128:c * BL + (ch + 1) * 128] = \
                packed[:, ch * OUT_SIZE:(ch + 1) * OUT_SIZE]
    return full


def kernel(**inputs):
    from concourse.bass_utils import run_bass_kernel_spmd

    nc = _get_nc()
    in_maps = make_in_maps(**inputs)
    res = run_bass_kernel_spmd(nc, in_maps, list(range(N_CORES)))
    return unpack_out(res.results)

